# revision 3
# baseline (speedup 1.0000x reference)
"""Binarized linear: out = sign(x+eps) @ sign(w+eps).T on 8 trn2 cores.

Sharding: 4x2 grid. Core c=(r,s): rows x[r*2048:(r+1)*2048], rows
w[s*2048:(s+1)*2048]. Each core computes a [2048, 2048] output block; the
host concatenates. No collectives.

Per-core kernel (all arithmetic exact -> rel err 0 vs the f32 reference):
  - binarize x-shard to fp8e4m3 as +/-0.5 (DVE: (x>=0)-0.5), w-shard as +/-1
    (ACT Sign with +1e-20 bias, matching sign(v+1e-20)).
  - transpose to [K-on-partition] pair layout ENTIRELY on the DMA XBAR
    (dma_start(transpose=True) on f16 views: 2 adjacent fp8 k-values ride
    one 16-bit lane; our fp8 bytes always form normal f16 values so the
    move is bit-exact). The PE therefore runs a pure DoubleRow matmul
    stream -- no PE transposes at all (v1 spent ~35-57us of PE on them).
  - DR matmuls run perf_mode=DoubleRowSwInterleave on the interleaved pair
    layout (hw deinterleaves; reads the stationary m axis reversed -- the
    host un-flips each 128-row output block). Measured cadence: 259ns per
    K=256 x 512-wide pass; the ~46ns/pass gap vs the 213ns streaming
    theory is a fixed per-instruction bubble (PSUM-bank alternation and
    2-bank outputs were measured/rejected: no effect / ISA-illegal).
  - PE floor: 1024 passes x 259ns = 265us/core. DMA floor: 64MB input at
    ~350-380GB/s (16-engine ceiling, ~24GB/s each) = ~175us + 16MB XBAR
    (~160GB/s) + 8MB output stores; engines stay under the PE span.
  - eviction scales PSUM by 2 (products are +/-0.5) -> exact even
    integers stored f16 (exact up to 4096; |out| <= 4096).

Schedule ("debt" order -- keeps PE dense while only ~4MB must land before
the first matmul; v1 waited ~50us for 20MB):
  - queues: inputs on the SP (sync) hw queue in arrival order; XBAR
    transposes + output stores on the Activation (scalar) hw queue.
  - arrival order: w-jb0, x0, w-jb1..3, x1, then x-blocks interleaved 1:1
    with w halves (chunk1 first, chunks 2/3 behind).
  - narrow phase (~t=16us): 8 n=128 pass-sets (ib 0/1 x jb 0..3) start as
    soon as w-jb0+x0 land, covering (ib0,jc0)+(ib1,jc0). Warm matmuls
    (no-dep identity passes) pad the PE clock gate open through startup.
  - solo phase: mm(ib,0) for ib 2..5, x-arrival paced, while chunk1 lands.
  - pair phase: mm(ib,0)+mm(ib,1) for ib 6..15 (8.3us PE per 2MB x block:
    PE-bound with supply slack for chunks 2/3).
  - debt: the owed mm(2..5, 1), then jc=2 and jc=3 sweeps (all resident).
  - out evictions (DVE) are emitted one set late so their PE-completion
    waits never head-of-line-block the DVE queue.
The Tile scheduler is fed PE timings scaled 2x (build_program patches
TRN2Spec) because the stock cost model prices DR fp8 matmuls at half
their measured hardware cost.
"""

import numpy as np

P = 128
GRID_I, GRID_J = 4, 2
N_CORES = 8
FULL_M, FULL_N, FULL_K = 8192, 4096, 4096
M_SH, N_SH = FULL_M // GRID_I, FULL_N // GRID_J  # 2048, 2048

_PROGRAM_CACHE = {}


def build_program(m_sh=M_SH, n_sh=N_SH, k=FULL_K, warmup=64, out_fp16=True):
    """Build (and cache) the per-core Bass program. Same SPMD program on all cores."""
    key = (m_sh, n_sh, k, warmup, out_fp16)
    if key in _PROGRAM_CACHE:
        return _PROGRAM_CACHE[key]

    from contextlib import ExitStack

    import concourse.bass as bass
    import concourse.mybir as mybir
    from concourse import bacc, tile
    from concourse.masks import make_identity

    # Feed the Tile scheduler PE timings that match measured hw (stock model
    # prices DR fp8 at 0.5 cyc/row; hw runs ~1.21 cyc/row incl the bubble).
    from concourse import hw_specs as _hw
    _hw.TRN2Spec.PE_CYCLE = 2.0 / 2.4
    _hw.TRN2Spec.PE_CYCLE_PSTATE_MID = 2.0 / 1.2
    _hw.TRN2Spec.PE_CYCLE_PSTATE_LOW = 2.0 / 0.65

    f32 = mybir.dt.float32
    f16 = mybir.dt.float16
    fp8 = mybir.dt.float8e4
    out_dt = f16 if out_fp16 else f32

    KT16 = k // (2 * P)  # 128-wide f16-pair k tiles (16)
    HH = k // 2          # half-row load width in f32 (2048)
    IB = m_sh // P       # 16 x blocks
    JBLK = 512
    JC = n_sh // JBLK    # 4 w chunks
    JB = n_sh // P       # 16 w j-blocks
    JB_PER_JC = JBLK // P

    DR = mybir.MatmulPerfMode.DoubleRowSwInterleave

    nc = bacc.Bacc("TRN2", target_bir_lowering=False, debug=False)
    xs = nc.dram_tensor("xs", [m_sh, k], f32, kind="ExternalInput").ap()
    ws = nc.dram_tensor("ws", [n_sh, k], f32, kind="ExternalInput").ap()
    out = nc.dram_tensor("out", [m_sh, n_sh], out_dt, kind="ExternalOutput").ap()

    with tile.TileContext(nc) as tc, ExitStack() as ctx:
        const_pool = ctx.enter_context(tc.tile_pool(name="const", bufs=1))
        stage_x = ctx.enter_context(tc.tile_pool(name="stagex", bufs=3))
        stage_w = ctx.enter_context(tc.tile_pool(name="stagew", bufs=3))
        xb8_pool = ctx.enter_context(tc.tile_pool(name="xb8", bufs=2))
        wb8_pool = ctx.enter_context(tc.tile_pool(name="wb8", bufs=2))
        xbt_pool = ctx.enter_context(tc.tile_pool(name="xbt", bufs=1))
        wbt_pool = ctx.enter_context(tc.tile_pool(name="wbt", bufs=1))
        out_pool = ctx.enter_context(tc.tile_pool(name="outp", bufs=3))
        outn_pool = ctx.enter_context(tc.tile_pool(name="outn", bufs=2))
        psum_mm = ctx.enter_context(tc.tile_pool(name="psmm", bufs=4, space="PSUM"))
        psum_nr = ctx.enter_context(tc.tile_pool(name="psnr", bufs=2, space="PSUM"))
        psum_wu = ctx.enter_context(tc.tile_pool(name="pswu", bufs=1, space="PSUM"))

        ident = const_pool.tile([P, P], fp8, tag="ident")
        make_identity(nc, ident)
        sign_bias = const_pool.tile([P, 1], f32, tag="sbias")
        nc.any.memset(sign_bias[:], 1e-20)

        warm_psum = psum_wu.tile([P, P], f32, tag="warm", name="warm") if warmup else None

        def warm(n):
            for _ in range(n):
                nc.tensor.matmul(warm_psum[:], lhsT=ident[:], rhs=ident[:],
                                 start=True, stop=True)

        if warmup:
            warm(warmup)

        # Resident transposed binarized operands, f16 pair layout, K on
        # partitions: f16 k-tile kp, partition p, byte b holds fp8 k value
        # 256*kp + 2p + b (consistent across x and w).
        xbT = [
            xbt_pool.tile([P, KT16, P], f16, tag=f"xbt{ib}", name=f"xbt{ib}")
            for ib in range(IB)
        ]
        wbT = [
            wbt_pool.tile([P, KT16, JBLK], f16, tag=f"wbt{jc}", name=f"wbt{jc}")
            for jc in range(JC)
        ]

        def bin_x(b8h, stgh):
            # (v >= 0) -> {1,0}; minus 0.5 -> +/-0.5. Matches sign(v+1e-20) up
            # to the measure-zero region (-1e-20, 0) that f32 randn never hits.
            nc.vector.tensor_scalar(
                b8h, stgh, 0.0, 0.5,
                mybir.AluOpType.is_ge, mybir.AluOpType.subtract,
            )

        def bin_w(b8h, stgh):
            nc.scalar.sign(b8h, stgh, bias=sign_bias[:])  # sign(w+1e-20) -> +/-1

        def prep_x(ib):
            """Load x block ib (two half-row DMAs), binarize (DVE), XBAR-
            transpose into xbT[ib] (scalar queue)."""
            b8 = xb8_pool.tile([P, k], fp8, tag="xb8", name="xb8")
            for h in range(2):
                stg = stage_x.tile([P, HH], f32, tag="stgx", name="stgx")
                nc.sync.dma_start(stg[:], xs[ib * P:(ib + 1) * P,
                                             h * HH:(h + 1) * HH])
                bin_x(b8[:, h * HH:(h + 1) * HH], stg[:])
            nc.scalar.dma_start(xbT[ib][:], b8.bitcast(f16), transpose=True)

        def prep_w_jb(jb):
            jc, sub = divmod(jb, JB_PER_JC)
            b8 = wb8_pool.tile([P, k], fp8, tag="wb8", name="wb8")
            for h in range(2):
                stg = stage_w.tile([P, HH], f32, tag="stgw", name="stgw")
                nc.sync.dma_start(stg[:], ws[jb * P:(jb + 1) * P,
                                             h * HH:(h + 1) * HH])
                bin_w(b8[:, h * HH:(h + 1) * HH], stg[:])
            nc.scalar.dma_start(wbT[jc][:, :, sub * P:(sub + 1) * P],
                                b8.bitcast(f16), transpose=True)

        # ---- load pacing state: interleave remaining x blocks and w jbs ----
        load_q = []
        for i in range(2, IB):
            load_q.append(("x", i))
            if 2 + i < JB:
                load_q.append(("w", 2 + i))
        for j in range(2 + IB, JB):
            load_q.append(("w", j))

        def advance_loads(n):
            for _ in range(n):
                if not load_q:
                    return
                kind, idx = load_q.pop(0)
                if kind == "x":
                    prep_x(idx)
                else:
                    prep_w_jb(idx)

        outq = []

        def emit_out(ps, ib, jc, width):
            ob_pool, tag = (out_pool, "ob") if width == JBLK else (outn_pool, "obn")
            ob = ob_pool.tile([P, width], out_dt, tag=tag, name=tag)
            # products are +/-0.5 (x) * +/-1 (w) = +/-0.5 -> scale by 2
            nc.vector.tensor_scalar_mul(ob[:], ps[:], 2.0)
            nc.scalar.dma_start(
                out[ib * P:(ib + 1) * P, jc * width:(jc + 1) * width], ob[:]
            )

        def flush_out():
            while outq:
                emit_out(*outq.pop(0))

        def mm(ib, jc):
            ps = psum_mm.tile([P, JBLK], f32, tag="ps", name="ps")
            for kp in range(KT16):
                lhsT = xbT[ib][:, kp, :].bitcast(fp8)
                rhs = wbT[jc][:, kp, :].bitcast(fp8).rearrange(
                    "p (n two) -> p two n", two=2)
                nc.tensor.matmul(ps[:], lhsT=lhsT, rhs=rhs,
                                 start=(kp == 0), stop=(kp == KT16 - 1),
                                 perf_mode=DR)
            if outq:
                emit_out(*outq.pop(0))
            outq.append((ps, ib, jc, JBLK))

        def narrow_mm(ib, jb):
            ps = psum_nr.tile([P, P], f32, tag="psn", name="psn")
            for kp in range(KT16):
                lhsT = xbT[ib][:, kp, :].bitcast(fp8)
                rhs = wbT[0][:, kp, jb * P:(jb + 1) * P].bitcast(fp8).rearrange(
                    "p (n two) -> p two n", two=2)
                nc.tensor.matmul(ps[:], lhsT=lhsT, rhs=rhs,
                                 start=(kp == 0), stop=(kp == KT16 - 1),
                                 perf_mode=DR)
            if outq:
                emit_out(*outq.pop(0))
            outq.append((ps, ib, jb, P))

        # ---- startup: minimal front-load, narrow sets start at ~4MB landed.
        prep_w_jb(0)
        prep_x(0)
        prep_w_jb(1)
        prep_w_jb(2)
        prep_w_jb(3)
        prep_x(1)
        # Warm filler holds the PE clock gate open while startup DMAs land.
        if warmup:
            warm(96)
        for ib in (0, 1):
            for jb in range(JB_PER_JC):
                narrow_mm(ib, jb)
                if warmup and ib == 0:
                    warm(8)
        # ---- solo phase: x-arrival paced, chunk1 streaming behind.
        for ib in range(2, 6):
            advance_loads(2)
            mm(ib, 0)
        # ---- pair phase.
        for ib in range(6, IB):
            advance_loads(2)
            mm(ib, 0)
            advance_loads(1)
            mm(ib, 1)
        # ---- debt: owed (0..5, 1).
        for ib in range(6):
            advance_loads(2)
            mm(ib, 1)
        # ---- jc 2/3 sweeps; finish remaining loads early in jc2.
        for jc in range(2, JC):
            for ib in range(IB):
                advance_loads(2)
                mm(ib, jc)
        flush_out()

    nc.compile()
    _PROGRAM_CACHE[key] = nc
    return nc


def kernel(x, weight):
    x = np.ascontiguousarray(np.asarray(x), dtype=np.float32)
    w = np.ascontiguousarray(np.asarray(weight), dtype=np.float32)
    assert x.shape == (FULL_M, FULL_K) and w.shape == (FULL_N, FULL_K)

    from concourse.bass_utils import run_bass_kernel_spmd

    nc = build_program()
    in_maps = []
    for c in range(N_CORES):
        r, s = divmod(c, GRID_J)
        in_maps.append({
            "xs": x[r * M_SH:(r + 1) * M_SH],
            "ws": w[s * N_SH:(s + 1) * N_SH],
        })
    res = run_bass_kernel_spmd(nc, in_maps, core_ids=list(range(N_CORES))).results
    outp = np.empty((FULL_M, FULL_N), dtype=np.float32)
    for c in range(N_CORES):
        r, s = divmod(c, GRID_J)
        blk = np.asarray(res[c]["out"], dtype=np.float32)
        # SwInterleave reads the stationary m axis reversed: un-flip each
        # 128-row output block.
        blk = blk.reshape(M_SH // P, P, N_SH)[:, ::-1, :].reshape(M_SH, N_SH)
        outp[r * M_SH:(r + 1) * M_SH, s * N_SH:(s + 1) * N_SH] = blk
    return outp
